# revision 3
# baseline (speedup 1.0000x reference)
"""HardQuadLoss Trainium2 kernel: hardest-positive/hardest-negative margin loss.

Strategy (8 NeuronCores, data-parallel over rows):
 - Host: sort rows by class (stable), so same-class columns are contiguous
   blocks. Each core owns 1024 sorted rows. Per-core column ROTATION places
   the core's own rows at columns [512, 1536), so the class-block "band" of
   every 128-row tile falls in a compile-time-static 1536-column window ->
   a single SPMD program works for all cores.
 - Device per core, per 128-row tile: PE computes h' = <x_i, x_j> - sq_j/2
   (float32r, 3 accumulated matmul passes; sq_j split hi/mid/lo rows so its
   precision survives f32r rounding) into PSUM. ACT copies PSUM->SBUF. DVE
   masks the band (+/-BIG trick on class equality) and computes, per row:
   max_same h' (-> dist_ap), max_diff h' (-> dist_an) and its argmax (the
   hardest-negative column) via the top-8 max / max_index instructions.
 - Host: unrotate indices, gather an[min_idx], relu/mean to the scalar loss.
"""

import sys

sys.path.insert(0, "/opt/trn_rl_repo")

import numpy as np

N = 8192
D = 256
NCORES = 8
SLAB = N // NCORES          # rows per core
RT = SLAB // 128            # 128-row tiles per core
NCHUNK = N // 512           # 512-col chunks per row
BIG = 4096.0                # band mask offset; > range of |h'| (~600)
ROT = 512                   # rotated position of each core's own rows
BAND_W = 1536               # static band window width (3 chunks)
MARGIN_SAME = 1.2
MARGIN_DIF = 0.3

_PROG_CACHE = {}


def _build_program(band_c0):
    """Build the SPMD Bass program. band_c0[r] = first 512-chunk of the static
    band window for row-tile r (same for all cores by construction)."""
    import concourse.bacc as bacc
    import concourse.mybir as mybir
    from concourse import tile

    F32 = mybir.dt.float32
    F32R = mybir.dt.float32r
    U32 = mybir.dt.uint32
    AL = mybir.AluOpType

    nc = bacc.Bacc(None, target_bir_lowering=False)

    with tile.TileContext(nc) as tc:
        with tc.tile_pool(name="dram", bufs=1, space="DRAM") as dram:
            d_xT = dram.tile([2, 128, N], F32R, kind="ExternalInput")
            d_sq3 = dram.tile([3, N], F32R, kind="ExternalInput")
            d_bct = dram.tile([128, N], F32, kind="ExternalInput")
            d_tsl = dram.tile([128, RT], F32, kind="ExternalInput")
            d_ssl = dram.tile([128, RT], F32, kind="ExternalInput")
            d_nh3 = dram.tile([3, 128], F32R, kind="ExternalInput")
            d_oap = dram.tile([128, RT], F32, kind="ExternalOutput")
            d_oan = dram.tile([128, RT], F32, kind="ExternalOutput")
            d_oix = dram.tile([128, RT], U32, kind="ExternalOutput")

            with tc.tile_pool(name="big", bufs=1) as bigp, \
                 tc.tile_pool(name="sn", bufs=2) as snp, \
                 tc.tile_pool(name="sm", bufs=2) as smp, \
                 tc.tile_pool(name="st", bufs=1) as stp, \
                 tc.tile_pool(name="ps", bufs=2, space="PSUM") as psp:
                xT0 = bigp.tile([128, N], F32R, tag="x0")
                xT1 = bigp.tile([128, N], F32R, tag="x1")
                bct = bigp.tile([128, N], F32, tag="bt")
                sq3 = stp.tile([3, N], F32R, tag="sq")
                nh3 = stp.tile([3, 128], F32R, tag="nh")
                tsl = stp.tile([128, RT], F32, tag="ts")
                ssl = stp.tile([128, RT], F32, tag="ss")
                nc.sync.dma_start(xT0[:], d_xT[0])
                nc.sync.dma_start(xT1[:], d_xT[1])
                nc.sync.dma_start(bct[:], d_bct[:])
                nc.sync.dma_start(sq3[:], d_sq3[:])
                nc.sync.dma_start(nh3[:], d_nh3[:])
                nc.sync.dma_start(tsl[:], d_tsl[:])
                nc.sync.dma_start(ssl[:], d_ssl[:])

                apacc = stp.tile([128, RT], F32, tag="apa")
                vall = stp.tile([128, RT], F32, tag="val")
                ixall = stp.tile([128, RT], U32, tag="ixa")

                for r in range(RT):
                    c0 = band_c0[r]
                    blo, bhi = c0 * 512, c0 * 512 + BAND_W
                    row0 = ROT + 128 * r
                    lhs0 = xT0[:, row0:row0 + 128]
                    lhs1 = xT1[:, row0:row0 + 128]

                    sn = snp.tile([128, N], F32, tag="sn")
                    for q in range(4):
                        hp = psp.tile([128, 2048], F32, tag="hp")
                        for c4 in range(4):
                            lo = c4 * 512
                            co = q * 2048 + lo
                            nc.tensor.matmul(hp[:, lo:lo + 512], lhs0,
                                             xT0[:, co:co + 512],
                                             start=True, stop=False)
                            nc.tensor.matmul(hp[:, lo:lo + 512], lhs1,
                                             xT1[:, co:co + 512],
                                             start=False, stop=False)
                            nc.tensor.matmul(hp[:, lo:lo + 512], nh3[:],
                                             sq3[:, co:co + 512],
                                             start=False, stop=True)
                        nc.scalar.copy(sn[:, q * 2048:(q + 1) * 2048], hp[:])

                    # band mask: mb = (class == class_i) * BIG; sn_band = h' - mb
                    mb = smp.tile([128, BAND_W], F32, tag="mb")
                    nc.vector.tensor_scalar(out=mb[:], in0=bct[:, blo:bhi],
                                            scalar1=tsl[:, r:r + 1], scalar2=BIG,
                                            op0=AL.is_equal, op1=AL.mult)
                    nc.vector.tensor_tensor(out=sn[:, blo:bhi],
                                            in0=sn[:, blo:bhi], in1=mb[:],
                                            op=AL.subtract)
                    # hardest positive: min over band of sn  (= min_same h' - BIG)
                    nc.vector.tensor_reduce(apacc[:, r:r + 1], sn[:, blo:bhi],
                                            axis=mybir.AxisListType.X, op=AL.min)
                    # hardest negative: global top-1 of sn + its column
                    mx8 = smp.tile([128, 8], F32, tag="mx8")
                    ix8 = smp.tile([128, 8], U32, tag="ix8")
                    nc.vector.max(mx8[:], sn[:])
                    nc.vector.max_index(ix8[:], mx8[:], sn[:])
                    nc.vector.tensor_copy(vall[:, r:r + 1], mx8[:, 0:1])
                    nc.vector.tensor_copy(ixall[:, r:r + 1], ix8[:, 0:1])

                # ap2 = sq_i - 2*(apmin + BIG);  an2 = sq_i - 2*vmax
                ap2 = stp.tile([128, RT], F32, tag="ap2")
                an2 = stp.tile([128, RT], F32, tag="an2")
                nc.vector.tensor_scalar(out=ap2[:], in0=apacc[:], scalar1=BIG,
                                        scalar2=-2.0, op0=AL.add, op1=AL.mult)
                nc.vector.tensor_tensor(out=ap2[:], in0=ap2[:], in1=ssl[:],
                                        op=AL.add)
                nc.vector.tensor_scalar(out=an2[:], in0=vall[:], scalar1=-2.0,
                                        scalar2=None, op0=AL.mult)
                nc.vector.tensor_tensor(out=an2[:], in0=an2[:], in1=ssl[:],
                                        op=AL.add)
                nc.sync.dma_start(d_oap[:], ap2[:])
                nc.sync.dma_start(d_oan[:], an2[:])
                nc.sync.dma_start(d_oix[:], ixall[:])

    names = dict(xT=d_xT.name, sq3=d_sq3.name, bct=d_bct.name, tsl=d_tsl.name,
                 ssl=d_ssl.name, nh3=d_nh3.name, oap=d_oap.name, oan=d_oan.name,
                 oix=d_oix.name)
    nc.compile()
    return nc, names


def _split3(v):
    """Split f32 vector into 3 addends, each exactly representable with a
    7-bit mantissa (survives any float32r rounding >= bf16 precision)."""
    v = v.astype(np.float32)
    h0 = (v.view(np.uint32) & np.uint32(0xFFFF0000)).view(np.float32)
    r0 = v - h0
    h1 = (r0.view(np.uint32) & np.uint32(0xFFFF0000)).view(np.float32)
    h2 = r0 - h1
    return np.stack([h0, h1, h2])


def _prepare(inputs, targets):
    """Sort rows by class, build per-core rotated in_maps. Returns
    (in_maps-ready host arrays, band_c0, perm)."""
    perm = np.argsort(targets, kind="stable")
    xs = np.ascontiguousarray(inputs[perm]).astype(np.float32)
    ts = targets[perm].astype(np.float32)
    sq = np.sum(xs.astype(np.float64) * xs.astype(np.float64), axis=1)
    sq = np.sum(xs * xs, axis=1, dtype=np.float32)
    xsT = np.ascontiguousarray(xs.T)  # [D, N]

    # static band windows: verify every (core, row-tile) class band fits
    band_c0 = []
    # class block boundaries in sorted order
    starts = np.searchsorted(ts, ts, side="left")
    ends = np.searchsorted(ts, ts, side="right")
    for r in range(RT):
        c0 = min((r + 1) >> 2, NCHUNK - 3)
        band_c0.append(c0)
    for c in range(NCORES):
        for r in range(RT):
            rows = slice(c * SLAB + r * 128, c * SLAB + r * 128 + 128)
            lo = starts[rows].min() - (c * SLAB - ROT)
            hi = ends[rows].max() - (c * SLAB - ROT)
            c0 = band_c0[r]
            assert c0 * 512 <= lo and hi <= c0 * 512 + BAND_W, \
                (c, r, lo, hi, c0)

    nh3 = np.full((3, 128), -0.5, np.float32)
    sq3 = _split3(sq)
    in_maps_host = []
    for c in range(NCORES):
        shift = ROT - c * SLAB
        xTc = np.roll(xsT, shift, axis=1)
        tgc = np.roll(ts, shift)
        sq3c = np.roll(sq3, shift, axis=1)
        rowsc = slice(c * SLAB, (c + 1) * SLAB)
        tslab = ts[rowsc].reshape(RT, 128).T  # [128, RT]
        sqslab = sq[rowsc].reshape(RT, 128).T
        in_maps_host.append(dict(
            xT=np.ascontiguousarray(xTc.reshape(2, 128, N)),
            sq3=np.ascontiguousarray(sq3c),
            bct=np.ascontiguousarray(np.broadcast_to(tgc, (128, N))),
            tsl=np.ascontiguousarray(tslab),
            ssl=np.ascontiguousarray(sqslab),
            nh3=nh3,
        ))
    return in_maps_host, band_c0


def _finish(results, names):
    """Host gather/unshard: assemble per-row stats, gather an[min_idx],
    compute the scalar loss."""
    ap2 = np.empty(N, np.float32)
    an2 = np.empty(N, np.float32)
    idx = np.empty(N, np.int64)
    for c in range(NCORES):
        r = results[c]
        rows = slice(c * SLAB, (c + 1) * SLAB)
        # device layout [128 partitions, RT] with row = r*128 + p
        ap2[rows] = r[names["oap"]].T.reshape(-1)
        an2[rows] = r[names["oan"]].T.reshape(-1)
        jrot = r[names["oix"]].T.reshape(-1).astype(np.int64)
        idx[rows] = (jrot + c * SLAB - ROT) % N
    dist_ap = np.sqrt(np.clip(ap2, 1e-12, None)).astype(np.float32)
    dist_an = np.sqrt(np.clip(an2, 1e-12, None)).astype(np.float32)
    dist_dif = dist_an[idx]
    loss_same = np.maximum(dist_ap - dist_an + MARGIN_SAME, 0.0).mean()
    loss_dif = np.maximum(dist_ap - dist_dif + MARGIN_DIF, 0.0).mean()
    return np.float32(loss_same + loss_dif)


def _install_trace_hook():
    """Shim antenv.axon_hooks (absent in this image) so bass_utils can NTFF-
    profile through the axon tunnel."""
    import types, importlib
    try:
        importlib.import_module("antenv.axon_hooks")
        return
    except ImportError:
        pass
    mod = types.ModuleType("antenv.axon_hooks")
    mod._hook = None

    def set_axon_ntff_profile_hook(h):
        mod._hook = h

    def get_axon_ntff_profile_hook():
        return mod._hook

    mod.set_axon_ntff_profile_hook = set_axon_ntff_profile_hook
    mod.get_axon_ntff_profile_hook = get_axon_ntff_profile_hook
    sys.modules["antenv.axon_hooks"] = mod
    try:
        from trn_agent_boot.trn_boot import _ntff_profile_via_ctypes
        hook = _ntff_profile_via_ctypes("/opt/axon/libaxon_pjrt.so")
        if hook is not None:
            set_axon_ntff_profile_hook(hook)
    except Exception:
        pass


def kernel(inputs, targets, _trace=False):
    from concourse.bass_utils import run_bass_kernel_spmd

    if _trace:
        _install_trace_hook()

    inputs = np.asarray(inputs, np.float32)
    targets_np = np.asarray(targets)
    in_maps_host, band_c0 = _prepare(inputs, targets_np)

    key = tuple(band_c0)
    if key not in _PROG_CACHE:
        _PROG_CACHE[key] = _build_program(band_c0)
    nc, names = _PROG_CACHE[key]

    in_maps = [{names[k]: v for k, v in m.items()} for m in in_maps_host]
    res = run_bass_kernel_spmd(nc, in_maps, core_ids=list(range(NCORES)),
                               trace=_trace)
    out = _finish(res.results, names)
    kernel.last_exec_time_ns = res.exec_time_ns
    return out


# revision 6
# speedup vs baseline: 1.2354x; 1.2354x over previous
"""HardQuadLoss Trainium2 kernel: hardest-positive/hardest-negative margin loss.

Strategy (8 NeuronCores, data-parallel over rows):
 - Host: sort rows by class (stable), so same-class columns are contiguous
   blocks. Each core owns 1024 sorted rows. Per-core column ROTATION places
   the core's own rows at columns [512, 1536), so the class-block "band" of
   every 128-row tile falls in a compile-time-static 1536-column window ->
   a single SPMD program works for all cores.
 - Device per core, per 128-row tile: PE computes h' = <x_i, x_j> - sq_j/2
   (float32r, 3 accumulated matmul passes; sq_j split hi/mid/lo rows so its
   precision survives f32r rounding) into PSUM. ACT copies PSUM->SBUF. DVE
   masks the band (+/-BIG trick on class equality) and computes, per row:
   max_same h' (-> dist_ap), max_diff h' (-> dist_an) and its argmax (the
   hardest-negative column) via the top-8 max / max_index instructions.
 - Host: unrotate indices, gather an[min_idx], relu/mean to the scalar loss.
"""

import sys

sys.path.insert(0, "/opt/trn_rl_repo")

import numpy as np

N = 8192
D = 256
NCORES = 8
SLAB = N // NCORES          # rows per core
RT = SLAB // 128            # 128-row tiles per core
NCHUNK = N // 512           # 512-col chunks per row
BIG = 4096.0                # band mask offset; > range of |h'| (~600)
ROT = 512                   # rotated position of each core's own rows
BAND_W = 1536               # static band window width (3 chunks)
MARGIN_SAME = 1.2
MARGIN_DIF = 0.3

_PROG_CACHE = {}


def _build_program(band_c0, band_w):
    """Build the SPMD Bass program. band_c0[r]/band_w[r] = first 512-chunk and
    width (cols) of the static band window for row-tile r (same for all cores
    by construction)."""
    import concourse.bacc as bacc
    import concourse.mybir as mybir
    from concourse import tile

    F32 = mybir.dt.float32
    F16 = mybir.dt.float16
    U32 = mybir.dt.uint32
    AL = mybir.AluOpType

    nc = bacc.Bacc(None, target_bir_lowering=False)

    with tile.TileContext(nc) as tc:
        with tc.tile_pool(name="dram", bufs=1, space="DRAM") as dram:
            d_xh = dram.tile([2, 128, N], F16, kind="ExternalInput")
            d_sq3 = dram.tile([3, N], F16, kind="ExternalInput")
            d_bct = dram.tile([128, N], F32, kind="ExternalInput")
            d_tsl = dram.tile([128, RT], F32, kind="ExternalInput")
            d_ssl = dram.tile([128, RT], F32, kind="ExternalInput")
            d_nh3 = dram.tile([3, 128], F16, kind="ExternalInput")
            d_oap = dram.tile([128, RT], F32, kind="ExternalOutput")
            d_oan = dram.tile([128, RT], F32, kind="ExternalOutput")
            d_oix = dram.tile([128, RT], U32, kind="ExternalOutput")

            with tc.tile_pool(name="big", bufs=1) as bigp, \
                 tc.tile_pool(name="sn", bufs=2) as snp, \
                 tc.tile_pool(name="sm", bufs=2) as smp, \
                 tc.tile_pool(name="st", bufs=1) as stp, \
                 tc.tile_pool(name="ps", bufs=2, space="PSUM") as psp:
                xh0 = bigp.tile([128, N], F16, tag="xh0")
                xh1 = bigp.tile([128, N], F16, tag="xh1")
                bct = bigp.tile([128, N], F32, tag="bt")
                sq3 = stp.tile([3, N], F16, tag="sq")
                nh3 = stp.tile([3, 128], F16, tag="nh")
                tsl = stp.tile([128, RT], F32, tag="ts")
                ssl = stp.tile([128, RT], F32, tag="ss")
                nc.sync.dma_start(xh0[:], d_xh[0])
                nc.sync.dma_start(xh1[:], d_xh[1])
                nc.sync.dma_start(bct[:], d_bct[:])
                nc.sync.dma_start(sq3[:], d_sq3[:])
                nc.sync.dma_start(nh3[:], d_nh3[:])
                nc.sync.dma_start(tsl[:], d_tsl[:])
                nc.sync.dma_start(ssl[:], d_ssl[:])

                apacc = stp.tile([128, RT], F32, tag="apa")
                vall = stp.tile([128, RT], F32, tag="val")
                ixall = stp.tile([128, RT], U32, tag="ixa")

                for r in range(RT):
                    blo = band_c0[r] * 512
                    bhi = blo + band_w[r]
                    row0 = ROT + 128 * r
                    lhs0 = xh0[:, row0:row0 + 128]
                    lhs1 = xh1[:, row0:row0 + 128]

                    sn = snp.tile([128, N], F32, tag="sn")
                    for q in range(4):
                        hp = psp.tile([128, 2048], F32, tag="hp")
                        for c4 in range(4):
                            lo = c4 * 512
                            co = q * 2048 + lo
                            nc.tensor.matmul(hp[:, lo:lo + 512], lhs0,
                                             xh0[:, co:co + 512],
                                             start=True, stop=False)
                            nc.tensor.matmul(hp[:, lo:lo + 512], lhs1,
                                             xh1[:, co:co + 512],
                                             start=False, stop=False)
                            nc.tensor.matmul(hp[:, lo:lo + 512], nh3[:],
                                             sq3[:, co:co + 512],
                                             start=False, stop=True)
                        nc.scalar.copy(sn[:, q * 2048:(q + 1) * 2048], hp[:])

                    # band mask: mb = (class == class_i) * BIG; sn_band = h' - mb
                    mb = smp.tile([128, band_w[r]], F32, tag="mb")
                    nc.vector.tensor_scalar(out=mb[:], in0=bct[:, blo:bhi],
                                            scalar1=tsl[:, r:r + 1], scalar2=BIG,
                                            op0=AL.is_equal, op1=AL.mult)
                    nc.vector.tensor_tensor(out=sn[:, blo:bhi],
                                            in0=sn[:, blo:bhi], in1=mb[:],
                                            op=AL.subtract)
                    # hardest positive: min over band of sn  (= min_same h' - BIG)
                    nc.vector.tensor_reduce(apacc[:, r:r + 1], sn[:, blo:bhi],
                                            axis=mybir.AxisListType.X, op=AL.min)
                    # hardest negative: global top-1 of sn + its column
                    mx8 = smp.tile([128, 8], F32, tag="mx8")
                    ix8 = smp.tile([128, 8], U32, tag="ix8")
                    nc.vector.max(mx8[:], sn[:])
                    nc.vector.max_index(ix8[:], mx8[:], sn[:])
                    nc.vector.tensor_copy(vall[:, r:r + 1], mx8[:, 0:1])
                    nc.vector.tensor_copy(ixall[:, r:r + 1], ix8[:, 0:1])

                # ap2 = sq_i - 2*(apmin + BIG);  an2 = sq_i - 2*vmax
                ap2 = stp.tile([128, RT], F32, tag="ap2")
                an2 = stp.tile([128, RT], F32, tag="an2")
                nc.vector.tensor_scalar(out=ap2[:], in0=apacc[:], scalar1=BIG,
                                        scalar2=-2.0, op0=AL.add, op1=AL.mult)
                nc.vector.tensor_tensor(out=ap2[:], in0=ap2[:], in1=ssl[:],
                                        op=AL.add)
                nc.vector.tensor_scalar(out=an2[:], in0=vall[:], scalar1=-2.0,
                                        scalar2=None, op0=AL.mult)
                nc.vector.tensor_tensor(out=an2[:], in0=an2[:], in1=ssl[:],
                                        op=AL.add)
                nc.sync.dma_start(d_oap[:], ap2[:])
                nc.sync.dma_start(d_oan[:], an2[:])
                nc.sync.dma_start(d_oix[:], ixall[:])

    names = dict(xh=d_xh.name, sq3=d_sq3.name, bct=d_bct.name, tsl=d_tsl.name,
                 ssl=d_ssl.name, nh3=d_nh3.name, oap=d_oap.name, oan=d_oan.name,
                 oix=d_oix.name)
    nc.compile()
    return nc, names


def _split3(v):
    """Split f32 vector into 3 fp16 addends: h0+h1+h2 ~= v to ~2^-33 rel."""
    v = v.astype(np.float32)
    h0 = v.astype(np.float16)
    r0 = v - h0.astype(np.float32)
    h1 = r0.astype(np.float16)
    h2 = (r0 - h1.astype(np.float32)).astype(np.float16)
    return np.stack([h0, h1, h2])


def _prepare(inputs, targets):
    """Sort rows by class, build per-core rotated in_maps. Returns
    (in_maps-ready host arrays, band_c0, perm)."""
    perm = np.argsort(targets, kind="stable")
    xs = np.ascontiguousarray(inputs[perm]).astype(np.float32)
    ts = targets[perm].astype(np.float32)
    sq = np.sum(xs * xs, axis=1, dtype=np.float32)
    xsT = np.ascontiguousarray(xs.T)  # [D, N]

    # static band windows (per row-tile, shared by all cores): smallest
    # 512-aligned window covering every core's class band for that tile
    starts = np.searchsorted(ts, ts, side="left")
    ends = np.searchsorted(ts, ts, side="right")
    band_c0, band_w = [], []
    for r in range(RT):
        lo, hi = N, 0
        for c in range(NCORES):
            rows = slice(c * SLAB + r * 128, c * SLAB + r * 128 + 128)
            lo = min(lo, starts[rows].min() - (c * SLAB - ROT))
            hi = max(hi, ends[rows].max() - (c * SLAB - ROT))
        c0 = lo // 512
        w = ((hi - c0 * 512 + 511) // 512) * 512
        assert 0 <= c0 and c0 * 512 + w <= N and w <= 1536, (r, lo, hi)
        band_c0.append(int(c0))
        band_w.append(int(w))

    nh3 = np.full((3, 128), -0.5, np.float16)
    sq3 = _split3(sq)
    xsT16 = xsT.astype(np.float16)
    in_maps_host = []
    for c in range(NCORES):
        shift = ROT - c * SLAB
        xTc = np.roll(xsT16, shift, axis=1)
        tgc = np.roll(ts, shift)
        sq3c = np.roll(sq3, shift, axis=1)
        rowsc = slice(c * SLAB, (c + 1) * SLAB)
        tslab = ts[rowsc].reshape(RT, 128).T  # [128, RT]
        sqslab = sq[rowsc].reshape(RT, 128).T
        in_maps_host.append(dict(
            xh=np.ascontiguousarray(xTc.reshape(2, 128, N)),
            sq3=np.ascontiguousarray(sq3c),
            bct=np.ascontiguousarray(np.broadcast_to(tgc, (128, N))),
            tsl=np.ascontiguousarray(tslab),
            ssl=np.ascontiguousarray(sqslab),
            nh3=nh3,
        ))
    return in_maps_host, band_c0, band_w


def _finish(results, names):
    """Host gather/unshard: assemble per-row stats, gather an[min_idx],
    compute the scalar loss."""
    ap2 = np.empty(N, np.float32)
    an2 = np.empty(N, np.float32)
    idx = np.empty(N, np.int64)
    for c in range(NCORES):
        r = results[c]
        rows = slice(c * SLAB, (c + 1) * SLAB)
        # device layout [128 partitions, RT] with row = r*128 + p
        ap2[rows] = r[names["oap"]].T.reshape(-1)
        an2[rows] = r[names["oan"]].T.reshape(-1)
        jrot = r[names["oix"]].T.reshape(-1).astype(np.int64)
        idx[rows] = (jrot + c * SLAB - ROT) % N
    dist_ap = np.sqrt(np.clip(ap2, 1e-12, None)).astype(np.float32)
    dist_an = np.sqrt(np.clip(an2, 1e-12, None)).astype(np.float32)
    dist_dif = dist_an[idx]
    loss_same = np.maximum(dist_ap - dist_an + MARGIN_SAME, 0.0).mean()
    loss_dif = np.maximum(dist_ap - dist_dif + MARGIN_DIF, 0.0).mean()
    return np.float32(loss_same + loss_dif)


def _install_trace_hook():
    """Shim antenv.axon_hooks (absent in this image) so bass_utils can NTFF-
    profile through the axon tunnel."""
    import types, importlib
    try:
        importlib.import_module("antenv.axon_hooks")
        return
    except ImportError:
        pass
    mod = types.ModuleType("antenv.axon_hooks")
    mod._hook = None

    def set_axon_ntff_profile_hook(h):
        mod._hook = h

    def get_axon_ntff_profile_hook():
        return mod._hook

    mod.set_axon_ntff_profile_hook = set_axon_ntff_profile_hook
    mod.get_axon_ntff_profile_hook = get_axon_ntff_profile_hook
    sys.modules["antenv.axon_hooks"] = mod
    try:
        from trn_agent_boot.trn_boot import _ntff_profile_via_ctypes
        hook = _ntff_profile_via_ctypes("/opt/axon/libaxon_pjrt.so")
        if hook is not None:
            set_axon_ntff_profile_hook(hook)
    except Exception:
        pass


def kernel(inputs, targets, _trace=False):
    from concourse.bass_utils import run_bass_kernel_spmd

    if _trace:
        _install_trace_hook()

    inputs = np.asarray(inputs, np.float32)
    targets_np = np.asarray(targets)
    in_maps_host, band_c0, band_w = _prepare(inputs, targets_np)

    key = (tuple(band_c0), tuple(band_w))
    if key not in _PROG_CACHE:
        _PROG_CACHE[key] = _build_program(band_c0, band_w)
    nc, names = _PROG_CACHE[key]

    in_maps = [{names[k]: v for k, v in m.items()} for m in in_maps_host]
    res = run_bass_kernel_spmd(nc, in_maps, core_ids=list(range(NCORES)),
                               trace=_trace)
    out = _finish(res.results, names)
    kernel.last_exec_time_ns = res.exec_time_ns
    return out


# revision 7
# speedup vs baseline: 1.4779x; 1.1962x over previous
"""HardQuadLoss Trainium2 kernel: hardest-positive/hardest-negative margin loss.

Strategy (8 NeuronCores, data-parallel over rows):
 - Host: sort rows by class (stable), so same-class columns are contiguous
   blocks. Each core owns 1024 sorted rows. Per-core column ROTATION places
   the core's own rows at columns [512, 1536), so the class-block "band" of
   every 128-row tile falls in a compile-time-static 1536-column window ->
   a single SPMD program works for all cores.
 - Device per core, per 128-row tile: PE computes h' = <x_i, x_j> - sq_j/2
   (float32r, 3 accumulated matmul passes; sq_j split hi/mid/lo rows so its
   precision survives f32r rounding) into PSUM. ACT copies PSUM->SBUF. DVE
   masks the band (+/-BIG trick on class equality) and computes, per row:
   max_same h' (-> dist_ap), max_diff h' (-> dist_an) and its argmax (the
   hardest-negative column) via the top-8 max / max_index instructions.
 - Host: unrotate indices, gather an[min_idx], relu/mean to the scalar loss.
"""

import sys

sys.path.insert(0, "/opt/trn_rl_repo")

import numpy as np

N = 8192
D = 256
NCORES = 8
SLAB = N // NCORES          # rows per core
RT = SLAB // 128            # 128-row tiles per core
NCHUNK = N // 512           # 512-col chunks per row
BIG = 4096.0                # band mask offset; > range of |h'| (~600)
ROT = 512                   # rotated position of each core's own rows
BAND_W = 1536               # static band window width (3 chunks)
MARGIN_SAME = 1.2
MARGIN_DIF = 0.3

_PROG_CACHE = {}


def _build_program(band_c0, band_w):
    """Build the SPMD Bass program. band_c0[r]/band_w[r] = first 512-chunk and
    width (cols) of the static band window for row-tile r (same for all cores
    by construction)."""
    import concourse.bacc as bacc
    import concourse.mybir as mybir
    from concourse import tile

    F32 = mybir.dt.float32
    F16 = mybir.dt.float16
    U32 = mybir.dt.uint32
    AL = mybir.AluOpType

    nc = bacc.Bacc(None, target_bir_lowering=False)

    with tile.TileContext(nc) as tc:
        with tc.tile_pool(name="dram", bufs=1, space="DRAM") as dram:
            d_xh = dram.tile([2, 128, N], F16, kind="ExternalInput")
            d_sq3 = dram.tile([3, N], F16, kind="ExternalInput")
            d_bct = dram.tile([128, N], F32, kind="ExternalInput")
            d_tsl = dram.tile([128, RT], F32, kind="ExternalInput")
            d_ssl = dram.tile([128, RT], F32, kind="ExternalInput")
            d_nh3 = dram.tile([3, 128], F16, kind="ExternalInput")
            d_oap = dram.tile([128, RT], F32, kind="ExternalOutput")
            d_oan = dram.tile([128, RT], F32, kind="ExternalOutput")
            d_oix = dram.tile([128, RT], U32, kind="ExternalOutput")

            with tc.tile_pool(name="big", bufs=1) as bigp, \
                 tc.tile_pool(name="sn", bufs=3) as snp, \
                 tc.tile_pool(name="sm", bufs=2) as smp, \
                 tc.tile_pool(name="st", bufs=1) as stp, \
                 tc.tile_pool(name="ps", bufs=2, space="PSUM") as psp:
                xh0 = bigp.tile([128, N], F16, tag="xh0")
                xh1 = bigp.tile([128, N], F16, tag="xh1")
                bct = bigp.tile([128, N], F32, tag="bt")
                sq3 = stp.tile([3, N], F16, tag="sq")
                nh3 = stp.tile([3, 128], F16, tag="nh")
                tsl = stp.tile([128, RT], F32, tag="ts")
                ssl = stp.tile([128, RT], F32, tag="ss")
                for dc in range(4):
                    s = slice(dc * 2048, (dc + 1) * 2048)
                    nc.sync.dma_start(xh0[:, s], d_xh[0][:, s])
                    nc.sync.dma_start(xh1[:, s], d_xh[1][:, s])
                nc.sync.dma_start(bct[:], d_bct[:])
                nc.sync.dma_start(sq3[:], d_sq3[:])
                nc.sync.dma_start(nh3[:], d_nh3[:])
                nc.sync.dma_start(tsl[:], d_tsl[:])
                nc.sync.dma_start(ssl[:], d_ssl[:])

                apacc = stp.tile([128, RT], F32, tag="apa")
                vall = stp.tile([128, RT], F32, tag="val")
                ixall = stp.tile([128, RT], U32, tag="ixa")

                for r in range(RT):
                    blo = band_c0[r] * 512
                    bhi = blo + band_w[r]
                    row0 = ROT + 128 * r
                    lhs0 = xh0[:, row0:row0 + 128]
                    lhs1 = xh1[:, row0:row0 + 128]

                    sn = snp.tile([128, N], F32, tag="sn")
                    for q in range(4):
                        hp = psp.tile([128, 2048], F32, tag="hp")
                        for pi, (w, rhs) in enumerate([(lhs0, xh0), (lhs1, xh1),
                                                       (nh3, sq3)]):
                            for c4 in range(4):
                                lo = c4 * 512
                                co = q * 2048 + lo
                                nc.tensor.matmul(hp[:, lo:lo + 512], w,
                                                 rhs[:, co:co + 512],
                                                 start=(pi == 0), stop=(pi == 2))
                        nc.scalar.copy(sn[:, q * 2048:(q + 1) * 2048], hp[:])

                    # band mask: mb = (class == class_i) * BIG; sn_band = h' - mb
                    mb = smp.tile([128, band_w[r]], F32, tag="mb")
                    nc.vector.tensor_scalar(out=mb[:], in0=bct[:, blo:bhi],
                                            scalar1=tsl[:, r:r + 1], scalar2=BIG,
                                            op0=AL.is_equal, op1=AL.mult)
                    nc.vector.tensor_tensor(out=sn[:, blo:bhi],
                                            in0=sn[:, blo:bhi], in1=mb[:],
                                            op=AL.subtract)
                    # hardest positive: min over band of sn  (= min_same h' - BIG)
                    nc.vector.tensor_reduce(apacc[:, r:r + 1], sn[:, blo:bhi],
                                            axis=mybir.AxisListType.X, op=AL.min)
                    # hardest negative: global top-1 of sn + its column
                    mx8 = smp.tile([128, 8], F32, tag="mx8")
                    ix8 = smp.tile([128, 8], U32, tag="ix8")
                    nc.vector.max(mx8[:], sn[:])
                    nc.vector.max_index(ix8[:], mx8[:], sn[:])
                    nc.vector.tensor_copy(vall[:, r:r + 1], mx8[:, 0:1])
                    nc.vector.tensor_copy(ixall[:, r:r + 1], ix8[:, 0:1])

                # ap2 = sq_i - 2*(apmin + BIG);  an2 = sq_i - 2*vmax
                ap2 = stp.tile([128, RT], F32, tag="ap2")
                an2 = stp.tile([128, RT], F32, tag="an2")
                nc.vector.tensor_scalar(out=ap2[:], in0=apacc[:], scalar1=BIG,
                                        scalar2=-2.0, op0=AL.add, op1=AL.mult)
                nc.vector.tensor_tensor(out=ap2[:], in0=ap2[:], in1=ssl[:],
                                        op=AL.add)
                nc.vector.tensor_scalar(out=an2[:], in0=vall[:], scalar1=-2.0,
                                        scalar2=None, op0=AL.mult)
                nc.vector.tensor_tensor(out=an2[:], in0=an2[:], in1=ssl[:],
                                        op=AL.add)
                nc.sync.dma_start(d_oap[:], ap2[:])
                nc.sync.dma_start(d_oan[:], an2[:])
                nc.sync.dma_start(d_oix[:], ixall[:])

    names = dict(xh=d_xh.name, sq3=d_sq3.name, bct=d_bct.name, tsl=d_tsl.name,
                 ssl=d_ssl.name, nh3=d_nh3.name, oap=d_oap.name, oan=d_oan.name,
                 oix=d_oix.name)
    nc.compile()
    return nc, names


def _split3(v):
    """Split f32 vector into 3 fp16 addends: h0+h1+h2 ~= v to ~2^-33 rel."""
    v = v.astype(np.float32)
    h0 = v.astype(np.float16)
    r0 = v - h0.astype(np.float32)
    h1 = r0.astype(np.float16)
    h2 = (r0 - h1.astype(np.float32)).astype(np.float16)
    return np.stack([h0, h1, h2])


def _prepare(inputs, targets):
    """Sort rows by class, build per-core rotated in_maps. Returns
    (in_maps-ready host arrays, band_c0, perm)."""
    perm = np.argsort(targets, kind="stable")
    xs = np.ascontiguousarray(inputs[perm]).astype(np.float32)
    ts = targets[perm].astype(np.float32)
    sq = np.sum(xs * xs, axis=1, dtype=np.float32)
    xsT = np.ascontiguousarray(xs.T)  # [D, N]

    # static band windows (per row-tile, shared by all cores): smallest
    # 512-aligned window covering every core's class band for that tile
    starts = np.searchsorted(ts, ts, side="left")
    ends = np.searchsorted(ts, ts, side="right")
    band_c0, band_w = [], []
    for r in range(RT):
        lo, hi = N, 0
        for c in range(NCORES):
            rows = slice(c * SLAB + r * 128, c * SLAB + r * 128 + 128)
            lo = min(lo, starts[rows].min() - (c * SLAB - ROT))
            hi = max(hi, ends[rows].max() - (c * SLAB - ROT))
        c0 = lo // 512
        w = ((hi - c0 * 512 + 511) // 512) * 512
        assert 0 <= c0 and c0 * 512 + w <= N and w <= 1536, (r, lo, hi)
        band_c0.append(int(c0))
        band_w.append(int(w))

    nh3 = np.full((3, 128), -0.5, np.float16)
    sq3 = _split3(sq)
    xsT16 = xsT.astype(np.float16)
    in_maps_host = []
    for c in range(NCORES):
        shift = ROT - c * SLAB
        xTc = np.roll(xsT16, shift, axis=1)
        tgc = np.roll(ts, shift)
        sq3c = np.roll(sq3, shift, axis=1)
        rowsc = slice(c * SLAB, (c + 1) * SLAB)
        tslab = ts[rowsc].reshape(RT, 128).T  # [128, RT]
        sqslab = sq[rowsc].reshape(RT, 128).T
        in_maps_host.append(dict(
            xh=np.ascontiguousarray(xTc.reshape(2, 128, N)),
            sq3=np.ascontiguousarray(sq3c),
            bct=np.ascontiguousarray(np.broadcast_to(tgc, (128, N))),
            tsl=np.ascontiguousarray(tslab),
            ssl=np.ascontiguousarray(sqslab),
            nh3=nh3,
        ))
    return in_maps_host, band_c0, band_w


def _finish(results, names):
    """Host gather/unshard: assemble per-row stats, gather an[min_idx],
    compute the scalar loss."""
    ap2 = np.empty(N, np.float32)
    an2 = np.empty(N, np.float32)
    idx = np.empty(N, np.int64)
    for c in range(NCORES):
        r = results[c]
        rows = slice(c * SLAB, (c + 1) * SLAB)
        # device layout [128 partitions, RT] with row = r*128 + p
        ap2[rows] = r[names["oap"]].T.reshape(-1)
        an2[rows] = r[names["oan"]].T.reshape(-1)
        jrot = r[names["oix"]].T.reshape(-1).astype(np.int64)
        idx[rows] = (jrot + c * SLAB - ROT) % N
    dist_ap = np.sqrt(np.clip(ap2, 1e-12, None)).astype(np.float32)
    dist_an = np.sqrt(np.clip(an2, 1e-12, None)).astype(np.float32)
    dist_dif = dist_an[idx]
    loss_same = np.maximum(dist_ap - dist_an + MARGIN_SAME, 0.0).mean()
    loss_dif = np.maximum(dist_ap - dist_dif + MARGIN_DIF, 0.0).mean()
    return np.float32(loss_same + loss_dif)


def _install_trace_hook():
    """Shim antenv.axon_hooks (absent in this image) so bass_utils can NTFF-
    profile through the axon tunnel."""
    import types, importlib
    try:
        importlib.import_module("antenv.axon_hooks")
        return
    except ImportError:
        pass
    mod = types.ModuleType("antenv.axon_hooks")
    mod._hook = None

    def set_axon_ntff_profile_hook(h):
        mod._hook = h

    def get_axon_ntff_profile_hook():
        return mod._hook

    mod.set_axon_ntff_profile_hook = set_axon_ntff_profile_hook
    mod.get_axon_ntff_profile_hook = get_axon_ntff_profile_hook
    sys.modules["antenv.axon_hooks"] = mod
    try:
        from trn_agent_boot.trn_boot import _ntff_profile_via_ctypes
        hook = _ntff_profile_via_ctypes("/opt/axon/libaxon_pjrt.so")
        if hook is not None:
            set_axon_ntff_profile_hook(hook)
    except Exception:
        pass


def kernel(inputs, targets, _trace=False):
    from concourse.bass_utils import run_bass_kernel_spmd

    if _trace:
        _install_trace_hook()

    inputs = np.asarray(inputs, np.float32)
    targets_np = np.asarray(targets)
    in_maps_host, band_c0, band_w = _prepare(inputs, targets_np)

    key = (tuple(band_c0), tuple(band_w))
    if key not in _PROG_CACHE:
        _PROG_CACHE[key] = _build_program(band_c0, band_w)
    nc, names = _PROG_CACHE[key]

    in_maps = [{names[k]: v for k, v in m.items()} for m in in_maps_host]
    res = run_bass_kernel_spmd(nc, in_maps, core_ids=list(range(NCORES)),
                               trace=_trace)
    out = _finish(res.results, names)
    kernel.last_exec_time_ns = res.exec_time_ns
    return out


# revision 8
# speedup vs baseline: 1.5102x; 1.0219x over previous
"""HardQuadLoss Trainium2 kernel: hardest-positive/hardest-negative margin loss.

Strategy (8 NeuronCores, data-parallel over rows):
 - Host: sort rows by class (stable), so same-class columns are contiguous
   blocks. Each core owns 1024 sorted rows. Per-core column ROTATION places
   the core's own rows at columns [512, 1536), so the class-block "band" of
   every 128-row tile falls in a compile-time-static 1536-column window ->
   a single SPMD program works for all cores.
 - Device per core, per 128-row tile: PE computes h' = <x_i, x_j> - sq_j/2
   (float32r, 3 accumulated matmul passes; sq_j split hi/mid/lo rows so its
   precision survives f32r rounding) into PSUM. ACT copies PSUM->SBUF. DVE
   masks the band (+/-BIG trick on class equality) and computes, per row:
   max_same h' (-> dist_ap), max_diff h' (-> dist_an) and its argmax (the
   hardest-negative column) via the top-8 max / max_index instructions.
 - Host: unrotate indices, gather an[min_idx], relu/mean to the scalar loss.
"""

import sys

sys.path.insert(0, "/opt/trn_rl_repo")

import numpy as np

N = 8192
D = 256
NCORES = 8
SLAB = N // NCORES          # rows per core
RT = SLAB // 128            # 128-row tiles per core
NCHUNK = N // 512           # 512-col chunks per row
BIG = 4096.0                # band mask offset; > range of |h'| (~600)
ROT = 512                   # rotated position of each core's own rows
BAND_W = 1536               # static band window width (3 chunks)
MARGIN_SAME = 1.2
MARGIN_DIF = 0.3

_PROG_CACHE = {}


def _build_program(band_c0, band_w):
    """Build the SPMD Bass program. band_c0[r]/band_w[r] = first 512-chunk and
    width (cols) of the static band window for row-tile r (same for all cores
    by construction)."""
    import concourse.bacc as bacc
    import concourse.mybir as mybir
    from concourse import tile

    F32 = mybir.dt.float32
    F16 = mybir.dt.float16
    U32 = mybir.dt.uint32
    AL = mybir.AluOpType

    nc = bacc.Bacc(None, target_bir_lowering=False)

    with tile.TileContext(nc) as tc:
        with tc.tile_pool(name="dram", bufs=1, space="DRAM") as dram:
            d_xh = dram.tile([2, 128, N], F16, kind="ExternalInput")
            d_sq3 = dram.tile([3, N], F16, kind="ExternalInput")
            d_bct = dram.tile([128, N], F32, kind="ExternalInput")
            d_tsl = dram.tile([128, RT], F32, kind="ExternalInput")
            d_ssl = dram.tile([128, RT], F32, kind="ExternalInput")
            d_nh3 = dram.tile([3, 128], F16, kind="ExternalInput")
            d_oap = dram.tile([128, RT], F32, kind="ExternalOutput")
            d_oan = dram.tile([128, RT], F32, kind="ExternalOutput")
            d_oix = dram.tile([128, RT], U32, kind="ExternalOutput")

            with tc.tile_pool(name="big", bufs=1) as bigp, \
                 tc.tile_pool(name="sn", bufs=3) as snp, \
                 tc.tile_pool(name="sm", bufs=2) as smp, \
                 tc.tile_pool(name="st", bufs=1) as stp, \
                 tc.tile_pool(name="ps", bufs=2, space="PSUM") as psp:
                xh0 = bigp.tile([128, N], F16, tag="xh0")
                xh1 = bigp.tile([128, N], F16, tag="xh1")
                bct = bigp.tile([128, N], F32, tag="bt")
                sq3 = stp.tile([3, N], F16, tag="sq")
                nh3 = stp.tile([3, 128], F16, tag="nh")
                tsl = stp.tile([128, RT], F32, tag="ts")
                ssl = stp.tile([128, RT], F32, tag="ss")
                for dc in range(8):
                    s = slice(dc * 1024, (dc + 1) * 1024)
                    nc.sync.dma_start(xh0[:, s], d_xh[0][:, s])
                    nc.sync.dma_start(xh1[:, s], d_xh[1][:, s])
                nc.sync.dma_start(bct[:], d_bct[:])
                nc.sync.dma_start(sq3[:], d_sq3[:])
                nc.sync.dma_start(nh3[:], d_nh3[:])
                nc.sync.dma_start(tsl[:], d_tsl[:])
                nc.sync.dma_start(ssl[:], d_ssl[:])

                apacc = stp.tile([128, RT], F32, tag="apa")
                vall = stp.tile([128, RT], F32, tag="val")
                ixall = stp.tile([128, RT], U32, tag="ixa")

                for r in range(RT):
                    blo = band_c0[r] * 512
                    bhi = blo + band_w[r]
                    row0 = ROT + 128 * r
                    lhs0 = xh0[:, row0:row0 + 128]
                    lhs1 = xh1[:, row0:row0 + 128]

                    sn = snp.tile([128, N], F32, tag="sn")
                    for q in range(4):
                        hp = psp.tile([128, 2048], F32, tag="hp")
                        for pi, (w, rhs) in enumerate([(lhs0, xh0), (lhs1, xh1),
                                                       (nh3, sq3)]):
                            for c4 in range(4):
                                lo = c4 * 512
                                co = q * 2048 + lo
                                nc.tensor.matmul(hp[:, lo:lo + 512], w,
                                                 rhs[:, co:co + 512],
                                                 start=(pi == 0), stop=(pi == 2))
                        nc.scalar.copy(sn[:, q * 2048:(q + 1) * 2048], hp[:])

                    # band mask: mb = (class == class_i) * BIG; sn_band = h' - mb
                    mb = smp.tile([128, band_w[r]], F32, tag="mb")
                    nc.vector.tensor_scalar(out=mb[:], in0=bct[:, blo:bhi],
                                            scalar1=tsl[:, r:r + 1], scalar2=BIG,
                                            op0=AL.is_equal, op1=AL.mult)
                    nc.vector.tensor_tensor(out=sn[:, blo:bhi],
                                            in0=sn[:, blo:bhi], in1=mb[:],
                                            op=AL.subtract)
                    # hardest positive: min over band of sn  (= min_same h' - BIG)
                    nc.vector.tensor_reduce(apacc[:, r:r + 1], sn[:, blo:bhi],
                                            axis=mybir.AxisListType.X, op=AL.min)
                    # hardest negative: global top-1 of sn + its column.
                    # Two half-row MAX8 scans (the 2nd half's scan can overlap
                    # the band ops / copies); elementwise max of the two top-8
                    # lists keeps slot 0 == the true global max, which is all
                    # FIND_INDEX8 and vall consume.
                    mx8a = smp.tile([128, 8], F32, tag="mx8a")
                    mx8b = smp.tile([128, 8], F32, tag="mx8b")
                    mx8 = smp.tile([128, 8], F32, tag="mx8")
                    ix8 = smp.tile([128, 8], U32, tag="ix8")
                    nc.vector.max(mx8b[:], sn[:, 4096:])
                    nc.vector.max(mx8a[:], sn[:, :4096])
                    nc.vector.tensor_tensor(out=mx8[:], in0=mx8a[:],
                                            in1=mx8b[:], op=AL.max)
                    nc.vector.max_index(ix8[:], mx8[:], sn[:])
                    nc.vector.tensor_copy(vall[:, r:r + 1], mx8[:, 0:1])
                    nc.vector.tensor_copy(ixall[:, r:r + 1], ix8[:, 0:1])

                # ap2 = sq_i - 2*(apmin + BIG);  an2 = sq_i - 2*vmax
                ap2 = stp.tile([128, RT], F32, tag="ap2")
                an2 = stp.tile([128, RT], F32, tag="an2")
                nc.vector.tensor_scalar(out=ap2[:], in0=apacc[:], scalar1=BIG,
                                        scalar2=-2.0, op0=AL.add, op1=AL.mult)
                nc.vector.tensor_tensor(out=ap2[:], in0=ap2[:], in1=ssl[:],
                                        op=AL.add)
                nc.vector.tensor_scalar(out=an2[:], in0=vall[:], scalar1=-2.0,
                                        scalar2=None, op0=AL.mult)
                nc.vector.tensor_tensor(out=an2[:], in0=an2[:], in1=ssl[:],
                                        op=AL.add)
                nc.sync.dma_start(d_oap[:], ap2[:])
                nc.sync.dma_start(d_oan[:], an2[:])
                nc.sync.dma_start(d_oix[:], ixall[:])

    names = dict(xh=d_xh.name, sq3=d_sq3.name, bct=d_bct.name, tsl=d_tsl.name,
                 ssl=d_ssl.name, nh3=d_nh3.name, oap=d_oap.name, oan=d_oan.name,
                 oix=d_oix.name)
    nc.compile()
    return nc, names


def _split3(v):
    """Split f32 vector into 3 fp16 addends: h0+h1+h2 ~= v to ~2^-33 rel."""
    v = v.astype(np.float32)
    h0 = v.astype(np.float16)
    r0 = v - h0.astype(np.float32)
    h1 = r0.astype(np.float16)
    h2 = (r0 - h1.astype(np.float32)).astype(np.float16)
    return np.stack([h0, h1, h2])


def _prepare(inputs, targets):
    """Sort rows by class, build per-core rotated in_maps. Returns
    (in_maps-ready host arrays, band_c0, perm)."""
    perm = np.argsort(targets, kind="stable")
    xs = np.ascontiguousarray(inputs[perm]).astype(np.float32)
    ts = targets[perm].astype(np.float32)
    sq = np.sum(xs * xs, axis=1, dtype=np.float32)
    xsT = np.ascontiguousarray(xs.T)  # [D, N]

    # static band windows (per row-tile, shared by all cores): smallest
    # 512-aligned window covering every core's class band for that tile
    starts = np.searchsorted(ts, ts, side="left")
    ends = np.searchsorted(ts, ts, side="right")
    band_c0, band_w = [], []
    for r in range(RT):
        lo, hi = N, 0
        for c in range(NCORES):
            rows = slice(c * SLAB + r * 128, c * SLAB + r * 128 + 128)
            lo = min(lo, starts[rows].min() - (c * SLAB - ROT))
            hi = max(hi, ends[rows].max() - (c * SLAB - ROT))
        c0 = lo // 512
        w = ((hi - c0 * 512 + 511) // 512) * 512
        assert 0 <= c0 and c0 * 512 + w <= N and w <= 1536, (r, lo, hi)
        band_c0.append(int(c0))
        band_w.append(int(w))

    nh3 = np.full((3, 128), -0.5, np.float16)
    sq3 = _split3(sq)
    xsT16 = xsT.astype(np.float16)
    in_maps_host = []
    for c in range(NCORES):
        shift = ROT - c * SLAB
        xTc = np.roll(xsT16, shift, axis=1)
        tgc = np.roll(ts, shift)
        sq3c = np.roll(sq3, shift, axis=1)
        rowsc = slice(c * SLAB, (c + 1) * SLAB)
        tslab = ts[rowsc].reshape(RT, 128).T  # [128, RT]
        sqslab = sq[rowsc].reshape(RT, 128).T
        in_maps_host.append(dict(
            xh=np.ascontiguousarray(xTc.reshape(2, 128, N)),
            sq3=np.ascontiguousarray(sq3c),
            bct=np.ascontiguousarray(np.broadcast_to(tgc, (128, N))),
            tsl=np.ascontiguousarray(tslab),
            ssl=np.ascontiguousarray(sqslab),
            nh3=nh3,
        ))
    return in_maps_host, band_c0, band_w


def _finish(results, names):
    """Host gather/unshard: assemble per-row stats, gather an[min_idx],
    compute the scalar loss."""
    ap2 = np.empty(N, np.float32)
    an2 = np.empty(N, np.float32)
    idx = np.empty(N, np.int64)
    for c in range(NCORES):
        r = results[c]
        rows = slice(c * SLAB, (c + 1) * SLAB)
        # device layout [128 partitions, RT] with row = r*128 + p
        ap2[rows] = r[names["oap"]].T.reshape(-1)
        an2[rows] = r[names["oan"]].T.reshape(-1)
        jrot = r[names["oix"]].T.reshape(-1).astype(np.int64)
        idx[rows] = (jrot + c * SLAB - ROT) % N
    dist_ap = np.sqrt(np.clip(ap2, 1e-12, None)).astype(np.float32)
    dist_an = np.sqrt(np.clip(an2, 1e-12, None)).astype(np.float32)
    dist_dif = dist_an[idx]
    loss_same = np.maximum(dist_ap - dist_an + MARGIN_SAME, 0.0).mean()
    loss_dif = np.maximum(dist_ap - dist_dif + MARGIN_DIF, 0.0).mean()
    return np.float32(loss_same + loss_dif)


def _install_trace_hook():
    """Shim antenv.axon_hooks (absent in this image) so bass_utils can NTFF-
    profile through the axon tunnel."""
    import types, importlib
    try:
        importlib.import_module("antenv.axon_hooks")
        return
    except ImportError:
        pass
    mod = types.ModuleType("antenv.axon_hooks")
    mod._hook = None

    def set_axon_ntff_profile_hook(h):
        mod._hook = h

    def get_axon_ntff_profile_hook():
        return mod._hook

    mod.set_axon_ntff_profile_hook = set_axon_ntff_profile_hook
    mod.get_axon_ntff_profile_hook = get_axon_ntff_profile_hook
    sys.modules["antenv.axon_hooks"] = mod
    try:
        from trn_agent_boot.trn_boot import _ntff_profile_via_ctypes
        hook = _ntff_profile_via_ctypes("/opt/axon/libaxon_pjrt.so")
        if hook is not None:
            set_axon_ntff_profile_hook(hook)
    except Exception:
        pass


def kernel(inputs, targets, _trace=False):
    from concourse.bass_utils import run_bass_kernel_spmd

    if _trace:
        _install_trace_hook()

    inputs = np.asarray(inputs, np.float32)
    targets_np = np.asarray(targets)
    in_maps_host, band_c0, band_w = _prepare(inputs, targets_np)

    key = (tuple(band_c0), tuple(band_w))
    if key not in _PROG_CACHE:
        _PROG_CACHE[key] = _build_program(band_c0, band_w)
    nc, names = _PROG_CACHE[key]

    in_maps = [{names[k]: v for k, v in m.items()} for m in in_maps_host]
    res = run_bass_kernel_spmd(nc, in_maps, core_ids=list(range(NCORES)),
                               trace=_trace)
    out = _finish(res.results, names)
    kernel.last_exec_time_ns = res.exec_time_ns
    return out
